# revision 11
# baseline (speedup 1.0000x reference)
"""CrossViewAttention3D Trainium2 kernel.

B=1, C=512, T=4, H=32, W=32 -> N=4096 tokens, 8 heads x head_dim 64.
Head-parallel across 8 NeuronCores: core h computes q/k/v projections for
its head, fused flash-style attention (S^T tiles -> exp on ACT -> AV
accumulate in PSUM, softmax denominator via a ones-column appended to
v^T), then the Wo column-slice partial out-projection.  Host sums the 8
partials and adds the output bias.

Engine strategy (from trace analysis: ACT's 128 exp calls are the hard
floor at ~1.1us each; PE must stay off its critical path):
 - Projections run in float32r (1 cycle/row at moving>=256) directly on
   the raw f32 input tiles -- no fp16 pre-casts at all.
 - S^T matmuls are K=64 row-packed pairs via tile_position (0,0)/(64,0);
   packed tiles stream concurrently (measured ~3ns apart), so a pair
   costs ~512 cycles.  q and k are duplicated across partitions 0-63 /
   64-127 (host duplicates the weight columns, so the projection matmuls
   produce both copies for free).
 - AV is also row-packed: each m-chunk's K=128 contraction splits into
   two K=64 halves accumulating into separate PSUM tiles (psA/psB), and
   the epilogue folds psA+psB on DVE.  Halves stream concurrently.
 - The out-projection (K=64) row-packs two c-chunks per slot; the folded
   numerator is written twice (partitions 0-63 and 64-127) to feed both.
 - xr streams on two DMA queues (sync + gpsimd) so pass 0 is not
   DMA-starved; xv and the output partials share the remaining capacity.

Self-contained: hardcodes all shapes; needs numpy + the installed
concourse/bass stack (axon-attached TRN2 cores via jax).
"""
import numpy as np

import concourse.tile as tile
from concourse import bacc, mybir
from concourse.bass_utils import run_bass_kernel_spmd
from concourse.masks import make_identity

f32 = mybir.dt.float32
f32r = mybir.dt.float32r
MMDT = mybir.dt.float16     # attention matmul operand dtype

B, C, T, H, W = 1, 512, 4, 32, 32
NHEADS = 8
D = C // NHEADS          # 64 head dim
P = 128                  # partitions
N = T * H * W            # 4096 tokens
NT = 512                 # n-tile (matmul moving dim)
NTILES = N // NT         # 8
CCH = C // P             # 4 c-chunks
MCH = N // P             # 32 m-chunks
NPAIR = MCH // 2         # 16 m-chunk pairs (row-packed S matmuls)
SCALE = float(D) ** -0.5  # 0.125

_EXP = mybir.ActivationFunctionType.Exp


def _build():
    nc = bacc.Bacc(None, target_bir_lowering=False, debug=False)
    xv = nc.dram_tensor("xv", [C, N], f32r, kind="ExternalInput")
    xr = nc.dram_tensor("xr", [C, N], f32r, kind="ExternalInput")
    # wq/wk carry the head weight columns duplicated (host sends [C, 2D])
    wq = nc.dram_tensor("wq", [C, 2 * D], f32r, kind="ExternalInput")
    wk = nc.dram_tensor("wk", [C, 2 * D], f32r, kind="ExternalInput")
    wv = nc.dram_tensor("wv", [C, D], f32r, kind="ExternalInput")
    bq = nc.dram_tensor("bq", [2 * D, 1], f32, kind="ExternalInput")
    bk = nc.dram_tensor("bk", [2 * D, 1], f32, kind="ExternalInput")
    bv = nc.dram_tensor("bv", [D, 1], f32, kind="ExternalInput")
    wo = nc.dram_tensor("wo", [D, C], f32, kind="ExternalInput")
    out = nc.dram_tensor("out", [C, N], f32, kind="ExternalOutput")

    xv_r = xv.rearrange("(o p) n -> p o n", p=P)
    xr_r = xr.rearrange("(o p) n -> p o n", p=P)

    with tile.TileContext(nc) as tc:
        with (
            tc.tile_pool(name="const", bufs=1) as const,
            tc.tile_pool(name="persist", bufs=1) as persist,
            tc.tile_pool(name="xvload", bufs=3) as xvload,
            tc.tile_pool(name="xrload", bufs=3) as xrload,
            tc.tile_pool(name="ptile", bufs=5) as ptile,
            tc.tile_pool(name="stage", bufs=4) as stage,
        ):
            # ---- exp table pre-warm: first ACT instruction triggers the
            # ~2.7us table load while the prologue DMAs stream ----
            warm = const.tile([1, 2], f32, tag="warm")
            nc.vector.memset(warm[:], 0.0)
            nc.scalar.activation(warm[0:1, 0:1], warm[0:1, 1:2], _EXP)

            # ---- weights / biases / identity ----
            wq_sb = const.tile([P, CCH, 2 * D], f32r, tag="wq")
            wk_sb = const.tile([P, CCH, 2 * D], f32r, tag="wk")
            wv_sb = const.tile([P, CCH, D], f32r, tag="wv")
            nc.gpsimd.dma_start(wq_sb[:], wq.rearrange("(o p) m -> p o m", p=P))
            nc.gpsimd.dma_start(wk_sb[:], wk.rearrange("(o p) m -> p o m", p=P))
            nc.gpsimd.dma_start(wv_sb[:], wv.rearrange("(o p) m -> p o m", p=P))
            wo_sb = const.tile([D, C], MMDT, tag="wo")
            nc.gpsimd.dma_start(wo_sb[:], wo[:])  # gpsimd DMA casts f32->f16
            bq_sb = const.tile([2 * D, 1], f32, tag="bq")
            bk_sb = const.tile([2 * D, 1], f32, tag="bk")
            bv_sb = const.tile([D, 1], f32, tag="bv")
            nc.gpsimd.dma_start(bq_sb[:], bq[:])
            nc.gpsimd.dma_start(bk_sb[:], bk[:])
            nc.gpsimd.dma_start(bv_sb[:], bv[:])

            ident = const.tile([D, D], MMDT, tag="ident")
            make_identity(nc, ident[:])
            # ---- persistent activations ----
            q_sb = persist.tile([P, N], MMDT, tag="q")    # rows 64:128 dup
            k_sb = persist.tile([P, N], MMDT, tag="k")
            v_sb = persist.tile([D, N], MMDT, tag="v")
            v1t = persist.tile([P, MCH, D + 1], MMDT, tag="v1t")
            ones_sb = const.tile([P, MCH], f32, tag="ones")
            nc.vector.memset(ones_sb[:], 1.0)
            nc.vector.tensor_copy(v1t[:, :, D], ones_sb[:])

            # ---- PSUM pools (flat, 8 banks total) ----
            with (
                tc.tile_pool(name="ps_s", bufs=2, space="PSUM") as ps_s,
                tc.tile_pool(name="ps_av", bufs=1, space="PSUM") as ps_av,
                tc.tile_pool(name="ps_op", bufs=2, space="PSUM") as ps_op,
            ):
                # AV accumulators: lo-half (psA) and hi-half (psB) of the
                # m-contraction, folded in the epilogue.
                psA = ps_av.tile([P, NT], f32, tag="avA", name="avA")
                psB = ps_av.tile([P, NT], f32, tag="avB", name="avB")

                # ---- helpers ----
                def load_x(dram_r, nt, tag, eng):
                    # per-c-chunk DMAs so projection matmuls start on the
                    # first 256KB instead of the full tile
                    pool = xvload if tag == "xv" else xrload
                    ns = slice(nt * NT, (nt + 1) * NT)
                    raw = pool.tile([P, CCH, NT], f32r, tag=tag,
                                    name=f"{tag}_{nt}")
                    for cc in range(CCH):
                        eng.dma_start(raw[:, cc], dram_r[:, cc, ns])
                    return raw

                def proj(dst, w_sb, b_sb, raw, nt, rows):
                    # f32r matmuls: 1 cycle/row at 512 moving cols, and no
                    # fp16 pre-cast of the input tile is needed
                    ns = slice(nt * NT, (nt + 1) * NT)
                    ps = ps_op.tile([P, NT], f32, tag="op", name=f"pj_{nt}")
                    for cc in range(CCH):
                        nc.tensor.matmul(ps[:rows],
                                         w_sb[:, cc], raw[:, cc],
                                         start=(cc == 0), stop=(cc == CCH - 1))
                    nc.vector.tensor_add(dst[:, ns], ps[:rows],
                                         b_sb[:, 0:1].to_broadcast([rows, NT]))

                def q_proj(nt):
                    raw = load_x(xv_r, nt, "xv", nc.gpsimd)
                    proj(q_sb, wq_sb, bq_sb, raw, nt, P)

                p_map = {}

                def emit_s(nt, j):
                    ns = slice(nt * NT, (nt + 1) * NT)
                    mca, mcb = 2 * j, 2 * j + 1
                    s_ps = ps_s.tile([P, 2, NT], f32, tag="s",
                                     name=f"s_{nt}_{j}")
                    nc.tensor.matmul(
                        s_ps[:, 0], k_sb[0:D, mca * P:(mca + 1) * P],
                        q_sb[0:D, ns], start=True, stop=True,
                        tile_position=(0, 0))
                    nc.tensor.matmul(
                        s_ps[:, 1], k_sb[D:P, mcb * P:(mcb + 1) * P],
                        q_sb[D:P, ns], start=True, stop=True,
                        tile_position=(64, 0))
                    p_t = ptile.tile([P, 2, NT], MMDT, tag="p",
                                     name=f"p_{nt}_{j}")
                    nc.scalar.activation(p_t[:], s_ps[:], _EXP, scale=SCALE)
                    p_map[(nt, j)] = p_t

                def emit_av(nt, j):
                    # row-packed AV: K=64 halves stream concurrently into
                    # separate accumulators
                    p_t = p_map.pop((nt, j))
                    start = (j == 0)
                    stop = (j == NPAIR - 1)
                    for sl, mc in ((0, 2 * j), (1, 2 * j + 1)):
                        nc.tensor.matmul(psA[0:D + 1], v1t[0:D, mc],
                                         p_t[0:D, sl], start=start and sl == 0,
                                         stop=stop and sl == 1,
                                         tile_position=(0, 0))
                        nc.tensor.matmul(psB[0:D + 1], v1t[D:P, mc],
                                         p_t[D:P, sl], start=start and sl == 0,
                                         stop=stop and sl == 1,
                                         tile_position=(64, 0))

                from collections import deque
                SKEW = 2
                av_q = deque()

                def push_s(nt, j):
                    emit_s(nt, j)
                    av_q.append((nt, j))
                    while len(av_q) > SKEW:
                        emit_av(*av_q.popleft())

                def drain_avs():
                    while av_q:
                        emit_av(*av_q.popleft())

                def epilogue_head(nt):
                    # fold the packed-AV halves; normalization by the softmax
                    # denominator commutes with Wo, so the out-projection
                    # consumes the UNNORMALIZED numerator and the divide
                    # happens on the projected tiles in epilogue_tail.
                    # DVE tensor_tensor cannot read two PSUM operands, so
                    # stage psB through SBUF first.
                    tmpB = stage.tile([D + 1, NT], f32, tag="tmpB")
                    nc.vector.tensor_copy(tmpB[:], psB[0:D + 1])
                    obar16 = stage.tile([D, NT], MMDT, tag="obar")
                    nc.vector.tensor_add(obar16[:], psA[0:D], tmpB[0:D])
                    den = stage.tile([1, NT], f32, tag="den")
                    nc.vector.tensor_add(den[:], psA[D:D + 1], tmpB[D:D + 1])
                    rec = stage.tile([1, NT], f32, tag="rec")
                    rscr = stage.tile([1, NT], f32, tag="rscr")
                    nc.vector.reciprocal_approx_accurate(rec[:], den[:],
                                                         rscr[:])
                    rb = stage.tile([P, NT], f32, tag="rb")
                    nc.gpsimd.partition_broadcast(rb[:], rec[:])
                    return obar16, rb

                def epilogue_tail(nt, obar16, rb):
                    ns = slice(nt * NT, (nt + 1) * NT)
                    for cc in range(CCH):
                        op_ps = ps_op.tile([P, NT], f32, tag="op",
                                           name=f"opj_{nt}_{cc}")
                        nc.tensor.matmul(op_ps[:],
                                         wo_sb[0:D, cc * P:(cc + 1) * P],
                                         obar16[:], start=True, stop=True)
                        ot = stage.tile([P, NT], f32, tag="ot")
                        nc.vector.tensor_mul(ot[:], op_ps[:], rb[:])
                        nc.sync.dma_start(out[cc * P:(cc + 1) * P, ns], ot[:])

                # ---- interleaved prologue + pass 0 ----
                # group g: load xr tile g (queues alternate sync/gpsimd),
                # project k/v, transpose v chunks; pass-0 S-pairs slot in
                # behind the k/v1t chunks they need so ACT starts filling
                # while the prologue is still streaming.  Only pass 0 runs
                # here: psA/psB can host a single accumulation group, so
                # passes must not interleave.  AV matmuls trail their S-pair
                # by SKEW slots globally so independent S work always sits
                # between dependent AVs in the PE FIFO.
                for g in range(NTILES):
                    eng = nc.sync if g % 2 == 0 else nc.gpsimd
                    raw = load_x(xr_r, g, "xr", eng)
                    proj(k_sb, wk_sb, bk_sb, raw, g, P)
                    proj(v_sb, wv_sb, bv_sb, raw, g, D)
                    for mc in range(4 * g, 4 * g + 4):
                        vt_ps = ps_op.tile([P, D], MMDT, tag="op",
                                           name=f"vt_{mc}")
                        nc.tensor.transpose(
                            vt_ps[:], v_sb[:, mc * P:(mc + 1) * P], ident[:])
                        nc.vector.tensor_copy(v1t[:, mc, 0:D], vt_ps[:])
                    if g == 0:
                        q_proj(0)
                    if g == 4:
                        q_proj(1)
                    push_s(0, 2 * g)
                    push_s(0, 2 * g + 1)

                # ---- remaining passes (sequential: one AV group at a time) ----
                pendings = deque()
                for nt in range(1, NTILES):
                    for j in range(NPAIR):
                        push_s(nt, j)
                        if j == 1:
                            # all of nt-1's AV pairs drained during j=0/1
                            pendings.append([nt - 1, *epilogue_head(nt - 1)])
                        if j == 4 and pendings:
                            epilogue_tail(*pendings.popleft())
                        if j == 12 and pendings:
                            epilogue_tail(*pendings.popleft())
                        if j == 8 and nt + 1 < NTILES:
                            q_proj(nt + 1)
                drain_avs()
                pendings.append([NTILES - 1, *epilogue_head(NTILES - 1)])
                while pendings:
                    epilogue_tail(*pendings.popleft())
    nc.compile()
    return nc


_cached_nc = None


def _get_nc():
    global _cached_nc
    if _cached_nc is None:
        _cached_nc = _build()
    return _cached_nc


def _make_in_maps(inp):
    xv = np.ascontiguousarray(inp["video_feat"].reshape(C, N), dtype=np.float32)
    xr = np.ascontiguousarray(inp["ref_feat"].reshape(C, N), dtype=np.float32)

    def dupc(a):  # duplicate columns: [C, D] -> [C, 2D]
        return np.ascontiguousarray(np.concatenate([a, a], axis=1),
                                    dtype=np.float32)

    in_maps = []
    for h in range(NHEADS):
        sl = slice(h * D, (h + 1) * D)
        wq_t = inp["Wq"][sl].T
        wk_t = inp["Wk"][sl].T
        in_maps.append({
            "xv": xv,
            "xr": xr,
            "wq": dupc(wq_t),
            "wk": dupc(wk_t),
            "wv": np.ascontiguousarray(inp["Wv"][sl].T, dtype=np.float32),
            "bq": np.ascontiguousarray(
                np.tile(inp["bq"][sl], 2).reshape(2 * D, 1), dtype=np.float32),
            "bk": np.ascontiguousarray(
                np.tile(inp["bk"][sl], 2).reshape(2 * D, 1), dtype=np.float32),
            "bv": np.ascontiguousarray(
                inp["bv"][sl].reshape(D, 1), dtype=np.float32),
            "wo": np.ascontiguousarray(inp["Wo"][:, sl].T, dtype=np.float32),
        })
    return in_maps


def run(inputs, **spmd_kwargs):
    """Run the kernel; returns (full_output, BassKernelResults)."""
    inp = {k: np.asarray(v) for k, v in inputs.items()}
    nc = _get_nc()
    res = run_bass_kernel_spmd(nc, _make_in_maps(inp),
                               list(range(NHEADS)), **spmd_kwargs)
    total = res.results[0]["out"].astype(np.float32).copy()
    for r in res.results[1:]:
        total += r["out"]
    total += np.asarray(inp["bo"], dtype=np.float32)[:, None]
    return total.reshape(B, C, T, H, W), res


def kernel(**inputs):
    out, _ = run(inputs)
    return out


# revision 12
# speedup vs baseline: 1.0931x; 1.0931x over previous
"""CrossViewAttention3D Trainium2 kernel.

B=1, C=512, T=4, H=32, W=32 -> N=4096 tokens, 8 heads x head_dim 64.
Head-parallel across 8 NeuronCores: core h computes q/k/v projections for
its head, fused flash-style attention (S^T tiles -> exp on ACT -> AV
accumulate in PSUM, softmax denominator via a ones-column appended to
v^T), then the Wo column-slice partial out-projection.  Host sums the 8
partials and adds the output bias.

Engine strategy (from trace analysis: ACT's 128 exp calls are the hard
floor at ~1.1us each; PE must stay off its critical path):
 - Projections run in float32r (1 cycle/row at moving>=256) directly on
   the raw f32 input tiles -- no fp16 pre-casts at all.
 - S^T matmuls are K=64 row-packed pairs via tile_position (0,0)/(64,0);
   packed tiles stream concurrently (measured ~3ns apart), so a pair
   costs ~512 cycles.  q and k are duplicated across partitions 0-63 /
   64-127 (host duplicates the weight columns, so the projection matmuls
   produce both copies for free).
 - AV is also row-packed: each m-chunk's K=128 contraction splits into
   two K=64 halves accumulating into separate PSUM tiles (psA/psB), and
   the epilogue folds psA+psB on DVE.  Halves stream concurrently.
 - The out-projection (K=64) row-packs two c-chunks per slot; the folded
   numerator is written twice (partitions 0-63 and 64-127) to feed both.
 - xr streams on two DMA queues (sync + gpsimd) so pass 0 is not
   DMA-starved; xv and the output partials share the remaining capacity.

Self-contained: hardcodes all shapes; needs numpy + the installed
concourse/bass stack (axon-attached TRN2 cores via jax).
"""
import numpy as np

import concourse.tile as tile
from concourse import bacc, mybir
from concourse.bass_utils import run_bass_kernel_spmd
from concourse.masks import make_identity

f32 = mybir.dt.float32
f32r = mybir.dt.float32r
MMDT = mybir.dt.float16     # attention matmul operand dtype

B, C, T, H, W = 1, 512, 4, 32, 32
NHEADS = 8
D = C // NHEADS          # 64 head dim
P = 128                  # partitions
N = T * H * W            # 4096 tokens
NT = 512                 # n-tile (matmul moving dim)
NTILES = N // NT         # 8
CCH = C // P             # 4 c-chunks
MCH = N // P             # 32 m-chunks
NPAIR = MCH // 2         # 16 m-chunk pairs (row-packed S matmuls)
SCALE = float(D) ** -0.5  # 0.125

_EXP = mybir.ActivationFunctionType.Exp


def _build():
    nc = bacc.Bacc(None, target_bir_lowering=False, debug=False)
    xv = nc.dram_tensor("xv", [C, N], f32r, kind="ExternalInput")
    xr = nc.dram_tensor("xr", [C, N], f32r, kind="ExternalInput")
    # wq/wk carry the head weight columns duplicated (host sends [C, 2D])
    wq = nc.dram_tensor("wq", [C, 2 * D], f32r, kind="ExternalInput")
    wk = nc.dram_tensor("wk", [C, 2 * D], f32r, kind="ExternalInput")
    wv = nc.dram_tensor("wv", [C, D], f32r, kind="ExternalInput")
    bq = nc.dram_tensor("bq", [2 * D, 1], f32, kind="ExternalInput")
    bk = nc.dram_tensor("bk", [2 * D, 1], f32, kind="ExternalInput")
    bv = nc.dram_tensor("bv", [D, 1], f32, kind="ExternalInput")
    wo = nc.dram_tensor("wo", [D, C], f32, kind="ExternalInput")
    out = nc.dram_tensor("out", [C, N], f32, kind="ExternalOutput")

    xv_r = xv.rearrange("(o p) n -> p o n", p=P)
    xr_r = xr.rearrange("(o p) n -> p o n", p=P)

    with tile.TileContext(nc) as tc:
        with (
            tc.tile_pool(name="const", bufs=1) as const,
            tc.tile_pool(name="persist", bufs=1) as persist,
            tc.tile_pool(name="xvload", bufs=3) as xvload,
            tc.tile_pool(name="xrload", bufs=3) as xrload,
            tc.tile_pool(name="ptile", bufs=5) as ptile,
            tc.tile_pool(name="stage", bufs=4) as stage,
        ):
            # ---- exp table pre-warm: first ACT instruction triggers the
            # ~2.7us table load while the prologue DMAs stream ----
            warm = const.tile([1, 2], f32, tag="warm")
            nc.vector.memset(warm[:], 0.0)
            nc.scalar.activation(warm[0:1, 0:1], warm[0:1, 1:2], _EXP)

            # ---- weights / biases / identity ----
            wq_sb = const.tile([P, CCH, 2 * D], f32r, tag="wq")
            wk_sb = const.tile([P, CCH, 2 * D], f32r, tag="wk")
            wv_sb = const.tile([P, CCH, D], f32r, tag="wv")
            nc.gpsimd.dma_start(wq_sb[:], wq.rearrange("(o p) m -> p o m", p=P))
            nc.gpsimd.dma_start(wk_sb[:], wk.rearrange("(o p) m -> p o m", p=P))
            nc.gpsimd.dma_start(wv_sb[:], wv.rearrange("(o p) m -> p o m", p=P))
            bq_sb = const.tile([2 * D, 1], f32, tag="bq")
            bk_sb = const.tile([2 * D, 1], f32, tag="bk")
            bv_sb = const.tile([D, 1], f32, tag="bv")
            nc.gpsimd.dma_start(bq_sb[:], bq[:])
            nc.gpsimd.dma_start(bk_sb[:], bk[:])
            nc.gpsimd.dma_start(bv_sb[:], bv[:])
            wo_sb = const.tile([D, C], MMDT, tag="wo")
            nc.gpsimd.dma_start(wo_sb[:], wo[:])  # gpsimd DMA casts f32->f16

            ident = const.tile([D, D], MMDT, tag="ident")
            make_identity(nc, ident[:])
            # ---- persistent activations ----
            q_sb = persist.tile([P, N], MMDT, tag="q")    # rows 64:128 dup
            k_sb = persist.tile([P, N], MMDT, tag="k")
            v_sb = persist.tile([D, N], MMDT, tag="v")
            v1t = persist.tile([P, MCH, D + 1], MMDT, tag="v1t")
            ones_sb = const.tile([P, MCH], f32, tag="ones")
            nc.vector.memset(ones_sb[:], 1.0)
            nc.vector.tensor_copy(v1t[:, :, D], ones_sb[:])

            # ---- PSUM pools (flat, 8 banks total) ----
            with (
                tc.tile_pool(name="ps_s", bufs=2, space="PSUM") as ps_s,
                tc.tile_pool(name="ps_av", bufs=1, space="PSUM") as ps_av,
                tc.tile_pool(name="ps_op", bufs=2, space="PSUM") as ps_op,
            ):
                # AV accumulators: lo-half (psA) and hi-half (psB) of the
                # m-contraction, folded in the epilogue.
                psA = ps_av.tile([P, NT], f32, tag="avA", name="avA")
                psB = ps_av.tile([P, NT], f32, tag="avB", name="avB")

                # ---- helpers ----
                def load_x(dram_r, nt, tag):
                    # per-c-chunk DMAs so projection matmuls start on the
                    # first 256KB instead of the full tile.  All x loads go
                    # on the sync HW-DGE queue: the gpsimd SW-DGE queue has
                    # multi-microsecond semaphore/drain latencies that stall
                    # consumers.
                    pool = xvload if tag == "xv" else xrload
                    ns = slice(nt * NT, (nt + 1) * NT)
                    raw = pool.tile([P, CCH, NT], f32r, tag=tag,
                                    name=f"{tag}_{nt}")
                    for cc in range(CCH):
                        nc.sync.dma_start(raw[:, cc], dram_r[:, cc, ns])
                    return raw

                def proj(dst, w_sb, b_sb, raw, nt, rows):
                    # f32r matmuls: 1 cycle/row at 512 moving cols, and no
                    # fp16 pre-cast of the input tile is needed
                    ns = slice(nt * NT, (nt + 1) * NT)
                    ps = ps_op.tile([P, NT], f32, tag="op", name=f"pj_{nt}")
                    for cc in range(CCH):
                        nc.tensor.matmul(ps[:rows],
                                         w_sb[:, cc], raw[:, cc],
                                         start=(cc == 0), stop=(cc == CCH - 1))
                    nc.vector.tensor_add(dst[:, ns], ps[:rows],
                                         b_sb[:, 0:1].to_broadcast([rows, NT]))

                def q_proj(nt):
                    raw = load_x(xv_r, nt, "xv")
                    proj(q_sb, wq_sb, bq_sb, raw, nt, P)

                p_map = {}

                def emit_s(nt, j):
                    ns = slice(nt * NT, (nt + 1) * NT)
                    mca, mcb = 2 * j, 2 * j + 1
                    s_ps = ps_s.tile([P, 2, NT], f32, tag="s",
                                     name=f"s_{nt}_{j}")
                    nc.tensor.matmul(
                        s_ps[:, 0], k_sb[0:D, mca * P:(mca + 1) * P],
                        q_sb[0:D, ns], start=True, stop=True,
                        tile_position=(0, 0))
                    nc.tensor.matmul(
                        s_ps[:, 1], k_sb[D:P, mcb * P:(mcb + 1) * P],
                        q_sb[D:P, ns], start=True, stop=True,
                        tile_position=(64, 0))
                    p_t = ptile.tile([P, 2, NT], MMDT, tag="p",
                                     name=f"p_{nt}_{j}")
                    nc.scalar.activation(p_t[:], s_ps[:], _EXP, scale=SCALE)
                    p_map[(nt, j)] = p_t

                def emit_av(nt, j):
                    # row-packed AV: K=64 halves stream concurrently into
                    # separate accumulators
                    p_t = p_map.pop((nt, j))
                    start = (j == 0)
                    stop = (j == NPAIR - 1)
                    for sl, mc in ((0, 2 * j), (1, 2 * j + 1)):
                        nc.tensor.matmul(psA[0:D + 1], v1t[0:D, mc],
                                         p_t[0:D, sl], start=start and sl == 0,
                                         stop=stop and sl == 1,
                                         tile_position=(0, 0))
                        nc.tensor.matmul(psB[0:D + 1], v1t[D:P, mc],
                                         p_t[D:P, sl], start=start and sl == 0,
                                         stop=stop and sl == 1,
                                         tile_position=(64, 0))

                from collections import deque
                SKEW = 2
                av_q = deque()

                def push_s(nt, j):
                    emit_s(nt, j)
                    av_q.append((nt, j))
                    while len(av_q) > SKEW:
                        emit_av(*av_q.popleft())

                def drain_avs():
                    while av_q:
                        emit_av(*av_q.popleft())

                def epilogue_head(nt):
                    # fold the packed-AV halves; normalization by the softmax
                    # denominator commutes with Wo, so the out-projection
                    # consumes the UNNORMALIZED numerator and the divide
                    # happens on the projected tiles in epilogue_tail.
                    # DVE tensor_tensor cannot read two PSUM operands, so
                    # stage psB through SBUF first.
                    tmpB = stage.tile([D + 1, NT], f32, tag="tmpB")
                    nc.vector.tensor_copy(tmpB[:], psB[0:D + 1])
                    obar16 = stage.tile([D, NT], MMDT, tag="obar")
                    nc.vector.tensor_add(obar16[:], psA[0:D], tmpB[0:D])
                    den = stage.tile([1, NT], f32, tag="den")
                    nc.vector.tensor_add(den[:], psA[D:D + 1], tmpB[D:D + 1])
                    rec = stage.tile([1, NT], f32, tag="rec")
                    rscr = stage.tile([1, NT], f32, tag="rscr")
                    nc.vector.reciprocal_approx_accurate(rec[:], den[:],
                                                         rscr[:])
                    rb = stage.tile([P, NT], f32, tag="rb")
                    nc.gpsimd.partition_broadcast(rb[:], rec[:])
                    return obar16, rb

                def epilogue_tail_cc(nt, obar16, rb, cc):
                    # one c-chunk of the out-projection: emitted in small
                    # pieces so the PE never queues a long block ahead of
                    # the S pairs that feed the (critical-path) exp stream
                    ns = slice(nt * NT, (nt + 1) * NT)
                    op_ps = ps_op.tile([P, NT], f32, tag="op",
                                       name=f"opj_{nt}_{cc}")
                    nc.tensor.matmul(op_ps[:],
                                     wo_sb[0:D, cc * P:(cc + 1) * P],
                                     obar16[:], start=True, stop=True)
                    ot = stage.tile([P, NT], f32, tag="ot")
                    nc.vector.tensor_mul(ot[:], op_ps[:], rb[:])
                    nc.sync.dma_start(out[cc * P:(cc + 1) * P, ns], ot[:])

                # ---- interleaved prologue + pass 0 ----
                # group g: load xr tile g (queues alternate sync/gpsimd),
                # project k/v, transpose v chunks; pass-0 S-pairs slot in
                # behind the k/v1t chunks they need so ACT starts filling
                # while the prologue is still streaming.  Only pass 0 runs
                # here: psA/psB can host a single accumulation group, so
                # passes must not interleave.  AV matmuls trail their S-pair
                # by SKEW slots globally so independent S work always sits
                # between dependent AVs in the PE FIFO.
                for g in range(NTILES):
                    raw = load_x(xr_r, g, "xr")
                    proj(k_sb, wk_sb, bk_sb, raw, g, P)
                    proj(v_sb, wv_sb, bv_sb, raw, g, D)
                    for mc in range(4 * g, 4 * g + 4):
                        vt_ps = ps_op.tile([P, D], MMDT, tag="op",
                                           name=f"vt_{mc}")
                        nc.tensor.transpose(
                            vt_ps[:], v_sb[:, mc * P:(mc + 1) * P], ident[:])
                        nc.vector.tensor_copy(v1t[:, mc, 0:D], vt_ps[:])
                    if g == 0:
                        q_proj(0)
                    if g == 4:
                        q_proj(1)
                    push_s(0, 2 * g)
                    push_s(0, 2 * g + 1)

                # ---- remaining passes (sequential: one AV group at a time) ----
                TAIL_AT = {4: 0, 7: 1, 10: 2, 13: 3}
                for nt in range(1, NTILES):
                    pend = None
                    for j in range(NPAIR):
                        push_s(nt, j)
                        if j == 1:
                            # all of nt-1's AV pairs drained during j=0/1
                            pend = [nt - 1, *epilogue_head(nt - 1)]
                        if j in TAIL_AT:
                            epilogue_tail_cc(*pend, TAIL_AT[j])
                        if j == 8 and nt + 1 < NTILES:
                            q_proj(nt + 1)
                drain_avs()
                pend = [NTILES - 1, *epilogue_head(NTILES - 1)]
                for cc in range(CCH):
                    epilogue_tail_cc(*pend, cc)
    nc.compile()
    return nc


_cached_nc = None


def _get_nc():
    global _cached_nc
    if _cached_nc is None:
        _cached_nc = _build()
    return _cached_nc


def _make_in_maps(inp):
    xv = np.ascontiguousarray(inp["video_feat"].reshape(C, N), dtype=np.float32)
    xr = np.ascontiguousarray(inp["ref_feat"].reshape(C, N), dtype=np.float32)

    def dupc(a):  # duplicate columns: [C, D] -> [C, 2D]
        return np.ascontiguousarray(np.concatenate([a, a], axis=1),
                                    dtype=np.float32)

    in_maps = []
    for h in range(NHEADS):
        sl = slice(h * D, (h + 1) * D)
        wq_t = inp["Wq"][sl].T
        wk_t = inp["Wk"][sl].T
        in_maps.append({
            "xv": xv,
            "xr": xr,
            "wq": dupc(wq_t),
            "wk": dupc(wk_t),
            "wv": np.ascontiguousarray(inp["Wv"][sl].T, dtype=np.float32),
            "bq": np.ascontiguousarray(
                np.tile(inp["bq"][sl], 2).reshape(2 * D, 1), dtype=np.float32),
            "bk": np.ascontiguousarray(
                np.tile(inp["bk"][sl], 2).reshape(2 * D, 1), dtype=np.float32),
            "bv": np.ascontiguousarray(
                inp["bv"][sl].reshape(D, 1), dtype=np.float32),
            "wo": np.ascontiguousarray(inp["Wo"][:, sl].T, dtype=np.float32),
        })
    return in_maps


def run(inputs, **spmd_kwargs):
    """Run the kernel; returns (full_output, BassKernelResults)."""
    inp = {k: np.asarray(v) for k, v in inputs.items()}
    nc = _get_nc()
    res = run_bass_kernel_spmd(nc, _make_in_maps(inp),
                               list(range(NHEADS)), **spmd_kwargs)
    total = res.results[0]["out"].astype(np.float32).copy()
    for r in res.results[1:]:
        total += r["out"]
    total += np.asarray(inp["bo"], dtype=np.float32)[:, None]
    return total.reshape(B, C, T, H, W), res


def kernel(**inputs):
    out, _ = run(inputs)
    return out


# revision 13
# speedup vs baseline: 1.1065x; 1.0122x over previous
"""CrossViewAttention3D Trainium2 kernel.

B=1, C=512, T=4, H=32, W=32 -> N=4096 tokens, 8 heads x head_dim 64.
Head-parallel across 8 NeuronCores: core h computes q/k/v projections for
its head, fused flash-style attention (S^T tiles -> exp on ACT -> AV
accumulate in PSUM, softmax denominator via a ones-column appended to
v^T), then the Wo column-slice partial out-projection.  Host sums the 8
partials and adds the output bias.

Engine strategy (from trace analysis: ACT's 128 exp calls are the hard
floor at ~1.1us each; PE must stay off its critical path):
 - Projections run in float32r (1 cycle/row at moving>=256) directly on
   the raw f32 input tiles -- no fp16 pre-casts at all.
 - S^T matmuls are K=64 row-packed pairs via tile_position (0,0)/(64,0);
   packed tiles stream concurrently (measured ~3ns apart), so a pair
   costs ~512 cycles.  q and k are duplicated across partitions 0-63 /
   64-127 (host duplicates the weight columns, so the projection matmuls
   produce both copies for free).
 - AV is also row-packed: each m-chunk's K=128 contraction splits into
   two K=64 halves accumulating into separate PSUM tiles (psA/psB), and
   the epilogue folds psA+psB on DVE.  Halves stream concurrently.
 - The out-projection (K=64) row-packs two c-chunks per slot; the folded
   numerator is written twice (partitions 0-63 and 64-127) to feed both.
 - xr streams on two DMA queues (sync + gpsimd) so pass 0 is not
   DMA-starved; xv and the output partials share the remaining capacity.

Self-contained: hardcodes all shapes; needs numpy + the installed
concourse/bass stack (axon-attached TRN2 cores via jax).
"""
import numpy as np

import concourse.tile as tile
from concourse import bacc, mybir
from concourse.bass_utils import run_bass_kernel_spmd
from concourse.masks import make_identity

f32 = mybir.dt.float32
f32r = mybir.dt.float32r
MMDT = mybir.dt.float16     # attention matmul operand dtype

B, C, T, H, W = 1, 512, 4, 32, 32
NHEADS = 8
D = C // NHEADS          # 64 head dim
P = 128                  # partitions
N = T * H * W            # 4096 tokens
NT = 512                 # n-tile (matmul moving dim)
NTILES = N // NT         # 8
CCH = C // P             # 4 c-chunks
MCH = N // P             # 32 m-chunks
NPAIR = MCH // 2         # 16 m-chunk pairs (row-packed S matmuls)
SCALE = float(D) ** -0.5  # 0.125

_EXP = mybir.ActivationFunctionType.Exp


def _build():
    nc = bacc.Bacc(None, target_bir_lowering=False, debug=False)
    xv = nc.dram_tensor("xv", [C, N], f32r, kind="ExternalInput")
    xr = nc.dram_tensor("xr", [C, N], f32r, kind="ExternalInput")
    # wq/wk carry the head weight columns duplicated (host sends [C, 2D])
    wq = nc.dram_tensor("wq", [C, 2 * D], f32r, kind="ExternalInput")
    wk = nc.dram_tensor("wk", [C, 2 * D], f32r, kind="ExternalInput")
    wv = nc.dram_tensor("wv", [C, D], f32r, kind="ExternalInput")
    bq = nc.dram_tensor("bq", [2 * D, 1], f32, kind="ExternalInput")
    bk = nc.dram_tensor("bk", [2 * D, 1], f32, kind="ExternalInput")
    bv = nc.dram_tensor("bv", [D, 1], f32, kind="ExternalInput")
    wo = nc.dram_tensor("wo", [D, C], f32, kind="ExternalInput")
    out = nc.dram_tensor("out", [C, N], f32, kind="ExternalOutput")

    xv_r = xv.rearrange("(o p) n -> p o n", p=P)
    xr_r = xr.rearrange("(o p) n -> p o n", p=P)

    with tile.TileContext(nc) as tc:
        with (
            tc.tile_pool(name="const", bufs=1) as const,
            tc.tile_pool(name="persist", bufs=1) as persist,
            tc.tile_pool(name="xvload", bufs=3) as xvload,
            tc.tile_pool(name="xrload", bufs=3) as xrload,
            tc.tile_pool(name="ptile", bufs=5) as ptile,
            tc.tile_pool(name="stage", bufs=4) as stage,
        ):
            # ---- exp table pre-warm: first ACT instruction triggers the
            # ~2.7us table load while the prologue DMAs stream ----
            warm = const.tile([1, 2], f32, tag="warm")
            nc.vector.memset(warm[:], 0.0)
            nc.scalar.activation(warm[0:1, 0:1], warm[0:1, 1:2], _EXP)

            # ---- weights / biases / identity ----
            wq_sb = const.tile([P, CCH, 2 * D], f32r, tag="wq")
            wk_sb = const.tile([P, CCH, 2 * D], f32r, tag="wk")
            wv_sb = const.tile([P, CCH, D], f32r, tag="wv")
            nc.sync.dma_start(wk_sb[:], wk.rearrange("(o p) m -> p o m", p=P))
            nc.sync.dma_start(wq_sb[:], wq.rearrange("(o p) m -> p o m", p=P))
            nc.sync.dma_start(wv_sb[:], wv.rearrange("(o p) m -> p o m", p=P))
            bq_sb = const.tile([2 * D, 1], f32, tag="bq")
            bk_sb = const.tile([2 * D, 1], f32, tag="bk")
            bv_sb = const.tile([D, 1], f32, tag="bv")
            nc.sync.dma_start(bq_sb[:], bq[:])
            nc.sync.dma_start(bk_sb[:], bk[:])
            nc.sync.dma_start(bv_sb[:], bv[:])
            wo_sb = const.tile([D, C], MMDT, tag="wo")
            nc.gpsimd.dma_start(wo_sb[:], wo[:])  # gpsimd DMA casts f32->f16

            ident = const.tile([D, D], MMDT, tag="ident")
            make_identity(nc, ident[:])
            # ---- persistent activations ----
            q_sb = persist.tile([P, N], MMDT, tag="q")    # rows 64:128 dup
            k_sb = persist.tile([P, N], MMDT, tag="k")
            v_sb = persist.tile([D, N], MMDT, tag="v")
            v1t = persist.tile([P, MCH, D + 1], MMDT, tag="v1t")
            ones_sb = const.tile([P, MCH], f32, tag="ones")
            nc.vector.memset(ones_sb[:], 1.0)
            nc.vector.tensor_copy(v1t[:, :, D], ones_sb[:])

            # ---- PSUM pools (flat, 8 banks total) ----
            with (
                tc.tile_pool(name="ps_s", bufs=2, space="PSUM") as ps_s,
                tc.tile_pool(name="ps_av", bufs=1, space="PSUM") as ps_av,
                tc.tile_pool(name="ps_op", bufs=2, space="PSUM") as ps_op,
            ):
                # AV accumulators: lo-half (psA) and hi-half (psB) of the
                # m-contraction, folded in the epilogue.
                psA = ps_av.tile([P, NT], f32, tag="avA", name="avA")
                psB = ps_av.tile([P, NT], f32, tag="avB", name="avB")

                # ---- helpers ----
                def load_x(dram_r, nt, tag):
                    # per-c-chunk DMAs so projection matmuls start on the
                    # first 256KB instead of the full tile.  All x loads go
                    # on the sync HW-DGE queue: the gpsimd SW-DGE queue has
                    # multi-microsecond semaphore/drain latencies that stall
                    # consumers.
                    pool = xvload if tag == "xv" else xrload
                    ns = slice(nt * NT, (nt + 1) * NT)
                    raw = pool.tile([P, CCH, NT], f32r, tag=tag,
                                    name=f"{tag}_{nt}")
                    for cc in range(CCH):
                        nc.sync.dma_start(raw[:, cc], dram_r[:, cc, ns])
                    return raw

                def proj(dst, w_sb, b_sb, raw, nt, rows, ccs=None,
                         ps=None):
                    # f32r matmuls: ~2 cycles/row at 512 moving cols, but no
                    # fp16 pre-cast of the input tile is needed.  ccs/ps let
                    # the caller emit the contraction in chunks so long PE
                    # blocks never delay the S pairs feeding the exp stream.
                    ns = slice(nt * NT, (nt + 1) * NT)
                    if ps is None:
                        ps = ps_op.tile([P, NT], f32, tag="op",
                                        name=f"pj_{nt}")
                    for cc in (range(CCH) if ccs is None else ccs):
                        nc.tensor.matmul(ps[:rows],
                                         w_sb[:, cc], raw[:, cc],
                                         start=(cc == 0), stop=(cc == CCH - 1))
                    if ccs is None or ccs[-1] == CCH - 1:
                        nc.vector.tensor_add(dst[:, ns], ps[:rows],
                                             b_sb[:, 0:1].to_broadcast(
                                                 [rows, NT]))
                    return ps

                def q_proj(nt, ccs=None, state=None):
                    if state is None:
                        state = [load_x(xv_r, nt, "xv"), None]
                    state[1] = proj(q_sb, wq_sb, bq_sb, state[0], nt, P,
                                    ccs=ccs, ps=state[1])
                    return state

                p_map = {}

                def emit_s(nt, j):
                    ns = slice(nt * NT, (nt + 1) * NT)
                    mca, mcb = 2 * j, 2 * j + 1
                    s_ps = ps_s.tile([P, 2, NT], f32, tag="s",
                                     name=f"s_{nt}_{j}")
                    nc.tensor.matmul(
                        s_ps[:, 0], k_sb[0:D, mca * P:(mca + 1) * P],
                        q_sb[0:D, ns], start=True, stop=True,
                        tile_position=(0, 0))
                    nc.tensor.matmul(
                        s_ps[:, 1], k_sb[D:P, mcb * P:(mcb + 1) * P],
                        q_sb[D:P, ns], start=True, stop=True,
                        tile_position=(64, 0))
                    p_t = ptile.tile([P, 2, NT], MMDT, tag="p",
                                     name=f"p_{nt}_{j}")
                    nc.scalar.activation(p_t[:], s_ps[:], _EXP, scale=SCALE)
                    p_map[(nt, j)] = p_t

                def emit_av(nt, j):
                    # row-packed AV: K=64 halves stream concurrently into
                    # separate accumulators
                    p_t = p_map.pop((nt, j))
                    start = (j == 0)
                    stop = (j == NPAIR - 1)
                    for sl, mc in ((0, 2 * j), (1, 2 * j + 1)):
                        nc.tensor.matmul(psA[0:D + 1], v1t[0:D, mc],
                                         p_t[0:D, sl], start=start and sl == 0,
                                         stop=stop and sl == 1,
                                         tile_position=(0, 0))
                        nc.tensor.matmul(psB[0:D + 1], v1t[D:P, mc],
                                         p_t[D:P, sl], start=start and sl == 0,
                                         stop=stop and sl == 1,
                                         tile_position=(64, 0))

                from collections import deque
                SKEW = 2
                av_q = deque()

                def push_s(nt, j):
                    emit_s(nt, j)
                    av_q.append((nt, j))
                    while len(av_q) > SKEW:
                        emit_av(*av_q.popleft())

                def drain_avs():
                    while av_q:
                        emit_av(*av_q.popleft())

                def epilogue_head(nt):
                    # fold the packed-AV halves; normalization by the softmax
                    # denominator commutes with Wo, so the out-projection
                    # consumes the UNNORMALIZED numerator and the divide
                    # happens on the projected tiles in epilogue_tail.
                    # DVE tensor_tensor cannot read two PSUM operands, so
                    # stage psB through SBUF first.
                    tmpB = stage.tile([D + 1, NT], f32, tag="tmpB")
                    nc.vector.tensor_copy(tmpB[:], psB[0:D + 1])
                    obar16 = stage.tile([D, NT], MMDT, tag="obar")
                    nc.vector.tensor_add(obar16[:], psA[0:D], tmpB[0:D])
                    den = stage.tile([1, NT], f32, tag="den")
                    nc.vector.tensor_add(den[:], psA[D:D + 1], tmpB[D:D + 1])
                    rec = stage.tile([1, NT], f32, tag="rec")
                    rscr = stage.tile([1, NT], f32, tag="rscr")
                    nc.vector.reciprocal_approx_accurate(rec[:], den[:],
                                                         rscr[:])
                    rb = stage.tile([P, NT], f32, tag="rb")
                    nc.gpsimd.partition_broadcast(rb[:], rec[:])
                    return obar16, rb

                def epilogue_tail_cc(nt, obar16, rb, cc):
                    # one c-chunk of the out-projection: emitted in small
                    # pieces so the PE never queues a long block ahead of
                    # the S pairs that feed the (critical-path) exp stream
                    ns = slice(nt * NT, (nt + 1) * NT)
                    op_ps = ps_op.tile([P, NT], f32, tag="op",
                                       name=f"opj_{nt}_{cc}")
                    nc.tensor.matmul(op_ps[:],
                                     wo_sb[0:D, cc * P:(cc + 1) * P],
                                     obar16[:], start=True, stop=True)
                    ot = stage.tile([P, NT], f32, tag="ot")
                    nc.vector.tensor_mul(ot[:], op_ps[:], rb[:])
                    nc.sync.dma_start(out[cc * P:(cc + 1) * P, ns], ot[:])

                # ---- interleaved prologue + pass 0 ----
                # group g: load xr tile g (queues alternate sync/gpsimd),
                # project k/v, transpose v chunks; pass-0 S-pairs slot in
                # behind the k/v1t chunks they need so ACT starts filling
                # while the prologue is still streaming.  Only pass 0 runs
                # here: psA/psB can host a single accumulation group, so
                # passes must not interleave.  AV matmuls trail their S-pair
                # by SKEW slots globally so independent S work always sits
                # between dependent AVs in the PE FIFO.
                for g in range(NTILES):
                    raw = load_x(xr_r, g, "xr")
                    proj(k_sb, wk_sb, bk_sb, raw, g, P)
                    if g == 0:
                        # q + the first S pairs go ahead of the v-projection:
                        # the exp stream (the critical path) starts ~3us
                        # earlier, and AV trails by SKEW so v1t arrives in
                        # time anyway
                        q_proj(0)
                        push_s(0, 0)
                        push_s(0, 1)
                    proj(v_sb, wv_sb, bv_sb, raw, g, D)
                    for mc in range(4 * g, 4 * g + 4):
                        vt_ps = ps_op.tile([P, D], MMDT, tag="op",
                                           name=f"vt_{mc}")
                        nc.tensor.transpose(
                            vt_ps[:], v_sb[:, mc * P:(mc + 1) * P], ident[:])
                        nc.vector.tensor_copy(v1t[:, mc, 0:D], vt_ps[:])
                    if g == 5:
                        q_proj(1)
                    if g > 0:
                        push_s(0, 2 * g)
                        push_s(0, 2 * g + 1)

                # ---- remaining passes (sequential: one AV group at a time) ----
                TAIL_AT = {4: 0, 7: 1, 10: 2, 13: 3}
                for nt in range(1, NTILES):
                    pend = None
                    for j in range(NPAIR):
                        push_s(nt, j)
                        if j == 1:
                            # all of nt-1's AV pairs drained during j=0/1
                            pend = [nt - 1, *epilogue_head(nt - 1)]
                        if j in TAIL_AT:
                            epilogue_tail_cc(*pend, TAIL_AT[j])
                        if j == 8 and nt + 1 < NTILES:
                            qstate = q_proj(nt + 1, ccs=[0, 1])
                        if j == 10 and nt + 1 < NTILES:
                            q_proj(nt + 1, ccs=[2, 3], state=qstate)
                drain_avs()
                pend = [NTILES - 1, *epilogue_head(NTILES - 1)]
                for cc in range(CCH):
                    epilogue_tail_cc(*pend, cc)
    nc.compile()
    return nc


_cached_nc = None


def _get_nc():
    global _cached_nc
    if _cached_nc is None:
        _cached_nc = _build()
    return _cached_nc


def _make_in_maps(inp):
    xv = np.ascontiguousarray(inp["video_feat"].reshape(C, N), dtype=np.float32)
    xr = np.ascontiguousarray(inp["ref_feat"].reshape(C, N), dtype=np.float32)

    def dupc(a):  # duplicate columns: [C, D] -> [C, 2D]
        return np.ascontiguousarray(np.concatenate([a, a], axis=1),
                                    dtype=np.float32)

    in_maps = []
    for h in range(NHEADS):
        sl = slice(h * D, (h + 1) * D)
        wq_t = inp["Wq"][sl].T
        wk_t = inp["Wk"][sl].T
        in_maps.append({
            "xv": xv,
            "xr": xr,
            "wq": dupc(wq_t),
            "wk": dupc(wk_t),
            "wv": np.ascontiguousarray(inp["Wv"][sl].T, dtype=np.float32),
            "bq": np.ascontiguousarray(
                np.tile(inp["bq"][sl], 2).reshape(2 * D, 1), dtype=np.float32),
            "bk": np.ascontiguousarray(
                np.tile(inp["bk"][sl], 2).reshape(2 * D, 1), dtype=np.float32),
            "bv": np.ascontiguousarray(
                inp["bv"][sl].reshape(D, 1), dtype=np.float32),
            "wo": np.ascontiguousarray(inp["Wo"][:, sl].T, dtype=np.float32),
        })
    return in_maps


def run(inputs, **spmd_kwargs):
    """Run the kernel; returns (full_output, BassKernelResults)."""
    inp = {k: np.asarray(v) for k, v in inputs.items()}
    nc = _get_nc()
    res = run_bass_kernel_spmd(nc, _make_in_maps(inp),
                               list(range(NHEADS)), **spmd_kwargs)
    total = res.results[0]["out"].astype(np.float32).copy()
    for r in res.results[1:]:
        total += r["out"]
    total += np.asarray(inp["bo"], dtype=np.float32)[:, None]
    return total.reshape(B, C, T, H, W), res


def kernel(**inputs):
    out, _ = run(inputs)
    return out
